# revision 20
# baseline (speedup 1.0000x reference)
"""Llama GQA attention (B=1, Q=1024, PAST=3072, HID=4096, NH=32, NKV=8, HD=128)
tensor-parallel over heads across 8 NeuronCores.

Per core c: kv head c, query heads 4c..4c+3. Each core computes its partial
o_proj contribution [1024, 4096] (fp16); the host sums the 8 partials.

Per-core design (fp16 datapath, f32 PSUM accumulation):
  - q/k/v projected TRANSPOSED: xT[d, seq] = W-tile.T @ hsT k-tiles, so no
    PE transposes for q/k (v is PE-transposed per 128-tile to [kv, d]).
    RoPE runs on [d, seq] PSUM tiles with 64-partition-shifted DVE ops
    (rotate-half lives on the partition dim); 1/sqrt(HD) is folded into Wq
    host-side so q and k share one rope table pair.
  - projection runs in two waves: k+v k-outer (4 accumulators, paced by
    per-superchunk hs DMA on three engine rings), then q0/q1; q2/q3 matmuls
    are pumped into attention passes 1-2 to fill PE gaps while ACT runs exp.
  - attention in 4 passes of (group g, head-pair): per kv tile kt one
    [128,1024] f32 PSUM scores tile (2 heads sharing the kT LDWEIGHTS), ONE
    exp activation over it, fp16 P. Fully-masked (g,kt) tiles are skipped;
    boundary tiles multiply a 0/1 fp16 mask after exp. Softmax runs without
    max-subtraction (constant shift cancels per row).
  - denominators: DVE fp16 2x-mode accumulation over kt; 1/D via N=1
    matmuls into a seq-on-partitions transpose, one DVE reciprocal
    [128,32], row-ified through identity matmuls and broadcast by a K=1
    matmul (no ACT involvement, so no table-set switches).
  - o_proj: out[seq, hid] tiles; per st the 4 attnT head-slice LDWEIGHTS
    are shared across all 8 hid blocks (8-bank PSUM ring); drains alternate
    DVE/ACT and the last head round pipelines drain+DMA per block.
"""

import math
import numpy as np
import ml_dtypes

import bass_rust
import concourse.bass as bass
import concourse.mybir as mybir
import concourse.tile as tile
from concourse.vector_clock import ScopedClock
from concourse.masks import make_identity
from concourse.bass_utils import run_bass_kernel_spmd

# ---------------------------------------------------------------------------
# Workaround: walrus in this image rejects >1 sem wait on CTRL-class
# instructions (Drain/NoOp). TileContext's tail drain waits on every touched
# logical processor. Split the waits across preceding sync-engine nops.
MAX_WAITS = 1


def _split_waits(nc, inst):
    si = inst.ins.sync_info
    if si is None:
        return
    waits = list(si.on_wait)
    if len(waits) <= MAX_WAITS:
        return
    inst.ins.sync_info = bass_rust.SyncInfo(
        on_wait=waits[:MAX_WAITS], on_update=list(si.on_update)
    )
    rest = waits[MAX_WAITS:]
    while rest:
        extra = nc.sync.nop(nofuse=True)
        extra.ins.sync_info = bass_rust.SyncInfo(on_wait=rest[:MAX_WAITS], on_update=[])
        rest = rest[MAX_WAITS:]


def _drain_and_barrier_split(self, tick_clock, wait_clock):
    nc = self.nc
    carrier = nc.sync.nop(nofuse=True)
    wait_clock.add_sem_waits(carrier.ins, ScopedClock({None: tick_clock.global_clock}))
    _split_waits(nc, carrier)
    nc.sync.drain()
    nc.all_engine_barrier()
    popped = nc._tile_sem_poison_stack.pop()
    assert popped is self._sem_poison
    nc.clear_and_free_semaphores(list(self.sems.allocated().values()))
    nc.all_engine_barrier()


tile.TileContext._drain_and_barrier = _drain_and_barrier_split
# ---------------------------------------------------------------------------

# ---------------------------------------------------------------------------
# General wait-cap legalization: hoist overflow waits onto engine-matched
# NoOps inserted immediately before the offender.
import json as _json

_CTRL_OPS = {"NoOp", "Drain", "EventSemaphore"}
_CAP_CTRL = 1
_CAP_OTHER = 1
_orig_to_json_bytes = bass.Bass.to_json_bytes


def _legalized_to_json_bytes(self, *a, **k):
    raw = _orig_to_json_bytes(self, *a, **k)
    m = _json.loads(raw)
    ctr = [0]
    changed = False
    for fn in m.get("functions", []):
        for blk in fn.get("blocks", []):
            insts = blk.get("instructions", [])
            out = []
            for ins in insts:
                si = ins.get("sync_info")
                if si:
                    waits = si.get("on_wait") or []
                    cap = _CAP_CTRL if ins.get("opcode") in _CTRL_OPS else _CAP_OTHER
                    if len(waits) > cap:
                        changed = True
                        rest = waits[:-cap]
                        si["on_wait"] = waits[-cap:]
                        while rest:
                            ctr[0] += 1
                            out.append({
                                "debug": ins.get("debug", 0),
                                "engine": ins["engine"],
                                "ins": [], "outs": [],
                                "name": f"{ins['name']}_lw{ctr[0]}",
                                "opcode": "NoOp",
                                "sync_info": {"on_wait": rest[:_CAP_CTRL],
                                              "on_update": []},
                            })
                            rest = rest[_CAP_CTRL:]
                out.append(ins)
            blk["instructions"] = out
    if not changed:
        return raw
    return _json.dumps(m).encode()


bass.Bass.to_json_bytes = _legalized_to_json_bytes
# ---------------------------------------------------------------------------


B, Q, PAST, HID = 1, 1024, 3072, 4096
NH, NKV, HD = 32, 8, 128
KV = PAST + Q           # 4096
NCORES = 8
HPC = NH // NCORES      # 4 query heads per core
ROPE_THETA = 10000.0
EXP_SHIFT = -11.0       # constant softmax shift (cancels exactly per row)

F32 = mybir.dt.float32
F16 = mybir.dt.float16
F8 = mybir.dt.float8e4
NPF16 = np.float16
NPF8 = ml_dtypes.float8_e4m3
W8SCALE = 1.0           # fp8 weight rescale (disabled: fp8 failed on HW)

N_KT = KV // 128        # 32 kv tiles
N_HK = HID // 128       # 32 hid k-tiles
GRP = 512
N_G = Q // GRP          # 2 groups
N_PV = PAST // 128      # 24 past-v tiles

LAST_RESULTS = None     # test harness reads exec_time_ns from here


def _build_program(kt_lists, boundary, nb):
    """kt_lists[g] = processed kv tiles for group g (fully-masked skipped);
    boundary[(g, kt)] = slot index into the maskb 0/1 tiles."""
    nc = bass.Bass()
    hst = nc.declare_dram_parameter("hst", [128, N_HK * Q], F16, isOutput=False)
    wqt = nc.declare_dram_parameter("wqt", [128, N_HK * HPC * 128], F16, isOutput=False)
    wkt = nc.declare_dram_parameter("wkt", [128, N_HK * 128], F16, isOutput=False)
    wvt = nc.declare_dram_parameter("wvt", [128, N_HK * 128], F16, isOutput=False)
    pastkt = nc.declare_dram_parameter("pastkt", [128, PAST], F16, isOutput=False)
    pastv = nc.declare_dram_parameter("pastv", [128, PAST], F16, isOutput=False)
    cost = nc.declare_dram_parameter("cost", [128, Q], F16, isOutput=False)
    sint = nc.declare_dram_parameter("sint", [128, Q], F16, isOutput=False)
    maskb = nc.declare_dram_parameter("maskb", [128, max(nb, 1) * GRP], F16,
                                      isOutput=False)
    wot = nc.declare_dram_parameter("wot", [128, HPC * HID], F16, isOutput=False)
    outp = nc.declare_dram_parameter("outp", [Q, HID], F16, isOutput=True)

    with tile.TileContext(nc) as tc:
        with (
            tc.tile_pool(name="const", bufs=1) as cpool,
            tc.tile_pool(name="kvres", bufs=1) as kvpool,
            tc.tile_pool(name="qat", bufs=1) as qat,
            tc.tile_pool(name="tbl", bufs=1) as tbl,
            tc.tile_pool(name="ptp", bufs=3) as ptp,
            tc.tile_pool(name="rt", bufs=1) as rt,
            tc.tile_pool(name="rcp", bufs=2) as rcp,
            tc.tile_pool(name="osb", bufs=4) as osbp,
        ):
            # K^T [128 d, KV]; V packed [128 kv-sub, kt*128 + d]
            kt_sb = kvpool.tile([128, KV], F16)
            v_sb = kvpool.tile([128, KV], F16)
            cos_sb = tbl.tile([128, Q], F16)
            sin_sb = tbl.tile([128, Q], F16)
            mb_sb = tbl.tile([128, max(nb, 1) * GRP], F16)

            qt = [qat.tile([128, Q], F16, tag=f"qt{h}", name=f"qt{h}")
                  for h in range(HPC)]
            atu = [qat.tile([128, Q], F16, tag=f"au{h}", name=f"au{h}")
                   for h in range(HPC)]
            dn = [qat.tile([128, GRP], F16, tag=f"dn{i}", name=f"dn{i}")
                  for i in range(2 * HPC)]

            hsw = tc.alloc_tile_pool(name="hsw", bufs=1)
            hs_sb = hsw.tile([128, N_HK * Q], F16)
            wq_sb = hsw.tile([128, N_HK * HPC * 128], F16)
            wk_sb = hsw.tile([128, N_HK * 128], F16)
            wv_sb = hsw.tile([128, N_HK * 128], F16)
            # DMA order == consumption order, issued round-robin over four
            # engine rings so fixed per-descriptor latencies overlap. wqt is
            # head-major so each q head can start as soon as its block lands.
            rings = [nc.sync, nc.scalar, nc.gpsimd]
            rr = [0]

            def dma(out, in_):
                rings[rr[0] % 3].dma_start(out, in_)
                rr[0] += 1

            SC = 4 * Q
            WH = N_HK * 128          # one head's wq block [128, 4096]
            # tiny k=0 lead chunks so the first matmuls start ASAP
            dma(wk_sb[:, 0:128], wkt[:, 0:128])
            dma(hs_sb[:, 0:Q], hst[:, 0:Q])
            dma(wv_sb[:, 0:128], wvt[:, 0:128])
            dma(wk_sb[:, 128:], wkt[:, 128:])
            dma(hs_sb[:, Q:SC], hst[:, Q:SC])
            dma(wv_sb[:, 128:], wvt[:, 128:])
            for j in range(1, 4):
                dma(hs_sb[:, j * SC:(j + 1) * SC], hst[:, j * SC:(j + 1) * SC])
            dma(wq_sb[:, 0:WH], wqt[:, 0:WH])
            dma(wq_sb[:, WH:2 * WH], wqt[:, WH:2 * WH])
            for j in range(4, 8):
                dma(hs_sb[:, j * SC:(j + 1) * SC], hst[:, j * SC:(j + 1) * SC])
            dma(wq_sb[:, 2 * WH:3 * WH], wqt[:, 2 * WH:3 * WH])
            dma(wq_sb[:, 3 * WH:4 * WH], wqt[:, 3 * WH:4 * WH])
            dma(cos_sb[:], cost[:])
            dma(sin_sb[:], sint[:])
            dma(kt_sb[:, :PAST], pastkt[:])
            dma(v_sb[:, : N_PV * 128], pastv[:])
            if nb:
                dma(mb_sb[:], maskb[:])

            ident = cpool.tile([128, 128], F16)
            make_identity(nc, ident[:])
            ones_col = cpool.tile([128, 1], F16)
            nc.vector.memset(ones_col[:], 1.0)
            ones_row = cpool.tile([1, 128], F32)
            nc.vector.memset(ones_row[:], 1.0)
            shift_sb = cpool.tile([128, 1], F32)
            nc.vector.memset(shift_sb[:], EXP_SHIFT)
            rcT = cpool.tile([128, 2 * 4 * HPC], F16)   # 1/D, seq on partitions
            # warm the exp table set while DMA streams in
            warm_in = cpool.tile([1, 8], F32)
            warm_out = cpool.tile([1, 8], F32)
            nc.vector.memset(warm_in[:], 0.0)
            nc.scalar.activation(warm_out[:], warm_in[:],
                                 mybir.ActivationFunctionType.Exp)


            def rope_half(dst, ps, g):
                """dst [128 d, 512] f16 <- rope(ps [128 d, 512] f32).

                rows 0:64 of sin_sb are pre-negated:
                  rot[0:64]  = ps[64:128] * sin[0:64]
                  rot[64:128]= ps[0:64]   * sin[64:128]
                """
                c = cos_sb[:, g * GRP:(g + 1) * GRP]
                s = sin_sb[:, g * GRP:(g + 1) * GRP]
                rot = rt.tile([128, GRP], F32, tag="rot", name="rot")
                nc.vector.tensor_mul(rot[0:64, :], ps[64:128, :], s[0:64, :])
                nc.vector.tensor_mul(rot[64:128, :], ps[0:64, :], s[64:128, :])
                cb = rt.tile([128, GRP], F32, tag="cb", name="cb")
                nc.vector.tensor_mul(cb[:], ps[:], c)
                nc.vector.tensor_add(dst, rot[:], cb[:])

            def wk_sl(k):
                return wk_sb[:, k * 128:(k + 1) * 128]

            def wv_sl(k):
                return wv_sb[:, k * 128:(k + 1) * 128]

            def wq_sl(h):
                return lambda k: wq_sb[:, h * N_HK * 128 + k * 128:
                                       h * N_HK * 128 + (k + 1) * 128]

            vts = rt.tile([128, Q], F16, tag="vt", name="vts")

            # ---- wave 1: k & v, k-outer (4 accumulators, DMA-paced) ----
            pj4a = tc.alloc_tile_pool(name="pj4a", bufs=4, space="PSUM")
            tgts = [(wk_sl, 0), (wk_sl, 1), (wv_sl, 0), (wv_sl, 1)]
            p4a = [pj4a.tile([128, GRP], F32, tag="p4a", name=f"p4a_{i}")
                   for i in range(4)]
            for k in range(N_HK):
                for i, (wfn, g) in enumerate(tgts):
                    nc.tensor.matmul(
                        p4a[i][:], wfn(k),
                        hs_sb[:, k * Q + g * GRP: k * Q + (g + 1) * GRP],
                        start=(k == 0), stop=(k == N_HK - 1))
            for g in range(N_G):
                rope_half(kt_sb[:, PAST + g * GRP: PAST + (g + 1) * GRP],
                          p4a[g][:], g)
            for g in range(N_G):
                nc.vector.tensor_copy(vts[:, g * GRP:(g + 1) * GRP],
                                      p4a[2 + g][:])
            pj4a.release()

            # ---- wave 2: q0, q1 sequential halves; ropes trail on DVE ----
            pj4b = tc.alloc_tile_pool(name="pj4b", bufs=4, space="PSUM")
            vtp = tc.alloc_tile_pool(name="vtp", bufs=2, space="PSUM")

            def vt_gen():
                for st in range(Q // 128):
                    tp = vtp.tile([128, 128], F16, tag="vtp", name="tp")
                    nc.tensor.transpose(
                        tp[:], vts[:, st * 128:(st + 1) * 128], ident[:])
                    yield
                    nc.vector.tensor_copy(
                        v_sb[:, (N_PV + st) * 128:(N_PV + st + 1) * 128], tp[:])

            def qk_gen(wslice_fn, dst_fn, pool, ptag):
                for g in range(N_G):
                    ps = pool.tile([128, GRP], F32, tag=ptag, name="qps")
                    for k in range(N_HK):
                        nc.tensor.matmul(
                            ps[:], wslice_fn(k),
                            hs_sb[:, k * Q + g * GRP: k * Q + (g + 1) * GRP],
                            start=(k == 0), stop=(k == N_HK - 1))
                        yield
                    rope_half(dst_fn(g), ps[:], g)

            def q_dst(h):
                return lambda g: qt[h][:, g * GRP:(g + 1) * GRP]

            def pump(gens, n):
                done = 0
                while gens and done < n:
                    try:
                        next(gens[0])
                        done += 1
                    except StopIteration:
                        gens.pop(0)

            w2 = [qk_gen(wq_sl(0), q_dst(0), pj4b, "p4b"), vt_gen(),
                  qk_gen(wq_sl(1), q_dst(1), pj4b, "p4b")]
            pump(w2, 10 ** 9)
            vtp.release()
            pj4b.release()

            # ---- normalization helpers (no ACT engine: 1/D via DVE
            # reciprocal on a seq-on-partitions transpose) ----
            def norm_reduce(g, pool, ttag):
                """rcT[:, g*16 + h*4 + c] = 1 / D(seq c*128+p) for head h."""
                dt = pool.tile([128, GRP], F32, tag=ttag, name="ndt")
                for h in range(HPC):
                    for c in range(4):
                        nc.tensor.matmul(
                            dt[:, h * 4 + c: h * 4 + c + 1],
                            dn[g * HPC + h][:, c * 128:(c + 1) * 128],
                            ones_col[:], start=True, stop=True)
                with nc.allow_low_precision(
                        reason="1/D in fp16: 0.05% rel err, budget is 2e-2"):
                    nc.vector.reciprocal(rcT[:, g * 16:(g + 1) * 16],
                                         dt[:, 0:16])

            def norm_apply_gen(g, pool, ttag):
                for h in range(HPC):
                    t_rc = pool.tile([128, GRP], F32, tag=ttag, name="nrc")
                    for c in range(4):
                        col = g * 16 + h * 4 + c
                        nc.tensor.matmul(
                            t_rc[0:1, c * 128:(c + 1) * 128],
                            rcT[:, col:col + 1], ident[:],
                            start=True, stop=True)
                        yield
                    rc_sb = rcp.tile([1, GRP], F32, tag="rc", name="rcs")
                    nc.vector.tensor_copy(rc_sb[:], t_rc[0:1, :])
                    t_bc = pool.tile([128, GRP], F32, tag=ttag, name="nbc")
                    nc.tensor.matmul(t_bc[:], ones_row[:], rc_sb[:],
                                     start=True, stop=True)
                    yield
                    nc.vector.tensor_mul(
                        atu[h][:, g * GRP:(g + 1) * GRP],
                        atu[h][:, g * GRP:(g + 1) * GRP], t_bc[:])

            # ---- attention passes ----
            pps = tc.alloc_tile_pool(name="pps", bufs=2, space="PSUM")
            scp = tc.alloc_tile_pool(name="scp", bufs=2, space="PSUM")
            att = tc.alloc_tile_pool(name="att", bufs=2, space="PSUM")

            def run_pass(g, ha, hb, bg, per_iter):
                kts = kt_lists[g]
                acc = [att.tile([128, GRP], F32, tag="acc",
                                name=f"acc{g}_{ha}_{jj}") for jj in range(2)]
                for i, kt in enumerate(kts):
                    s_ps = scp.tile([128, 2 * GRP], F32, tag="sc", name="sps")
                    for j, hh in enumerate((ha, hb)):
                        nc.tensor.matmul(
                            s_ps[:, j * GRP:(j + 1) * GRP],
                            kt_sb[:, kt * 128:(kt + 1) * 128],
                            qt[hh][:, g * GRP:(g + 1) * GRP],
                            start=True, stop=True)
                    pump(bg, per_iter)
                    pt = ptp.tile([128, 2 * GRP], F16, tag="pt", name="pt")
                    nc.scalar.activation(
                        pt[:], s_ps[:], mybir.ActivationFunctionType.Exp,
                        bias=shift_sb[:], scale=1.0)
                    sl = boundary.get((g, kt))
                    if sl is not None:
                        for j in range(2):
                            nc.vector.tensor_mul(
                                pt[:, j * GRP:(j + 1) * GRP],
                                pt[:, j * GRP:(j + 1) * GRP],
                                mb_sb[:, sl * GRP:(sl + 1) * GRP])
                    for j, hh in enumerate((ha, hb)):
                        half = pt[:, j * GRP:(j + 1) * GRP]
                        d = dn[g * HPC + hh]
                        if i == 0:
                            nc.vector.tensor_copy(d[:], half)
                        else:
                            nc.vector.tensor_add(d[:], d[:], half)
                        nc.tensor.matmul(
                            acc[j][:], v_sb[:, kt * 128:(kt + 1) * 128],
                            half, start=(i == 0), stop=(i == len(kts) - 1))
                for j, hh in enumerate((ha, hb)):
                    nc.vector.tensor_copy(
                        atu[hh][:, g * GRP:(g + 1) * GRP], acc[j][:])

            pending = [qk_gen(wq_sl(2), q_dst(2), pps, "p"),
                       qk_gen(wq_sl(3), q_dst(3), pps, "p")]
            run_pass(0, 0, 1, pending, 3)
            run_pass(1, 0, 1, pending, 3)
            pump(pending, 10 ** 9)
            # hs/wq/wkv no longer needed; free the space and stream in Wo
            hsw.release()
            wop = tc.alloc_tile_pool(name="wo", bufs=1)
            wo_sb = wop.tile([128, HPC * HID], F16)
            for i in range(8):
                s, e = i * (HPC * HID // 8), (i + 1) * (HPC * HID // 8)
                nc.sync.dma_start(wo_sb[:, s:e], wot[:, s:e])

            run_pass(0, 2, 3, [], 0)
            norm_reduce(0, pps, "p")
            na0 = [norm_apply_gen(0, pps, "p")]
            run_pass(1, 2, 3, na0, 1)
            pump(na0, 10 ** 9)
            att.release()
            scp.release()
            pps.release()

            # ---- tail: o_proj (LDW shared across the 8 n-blocks) ----
            tailp = tc.alloc_tile_pool(name="tailp", bufs=8, space="PSUM")

            def oproj_st(st, bg):
                tiles = [tailp.tile([128, GRP], F32, tag="o", name=f"o{n}")
                         for n in range(8)]
                for h in range(HPC):
                    for n in range(8):
                        nc.tensor.matmul(
                            tiles[n][:], atu[h][:, st * 128:(st + 1) * 128],
                            wo_sb[:, h * HID + n * GRP: h * HID + (n + 1) * GRP],
                            start=(h == 0), stop=(h == HPC - 1))
                        if h == HPC - 1:
                            ob = osbp.tile([128, GRP], F16, tag="ob", name="ob")
                            if (st + n) % 2 == 0:
                                nc.vector.tensor_copy(ob[:], tiles[n][:])
                            else:
                                nc.scalar.activation(
                                    ob[:], tiles[n][:],
                                    mybir.ActivationFunctionType.Copy)
                            nc.sync.dma_start(
                                outp[st * 128:(st + 1) * 128,
                                     n * GRP:(n + 1) * GRP], ob[:])
                    pump(bg, 2)

            oproj_st(0, [])
            norm_reduce(1, tailp, "o")
            na1 = [norm_apply_gen(1, tailp, "o")]
            oproj_st(1, na1)
            oproj_st(2, na1)
            pump(na1, 10 ** 9)
            for st in range(3, 8):
                oproj_st(st, [])
            tailp.release()
            wop.release()
    return nc


def _pack_ktiles(a, tile_rows=128):
    """[R, C] -> [128, (R//128)*C] with k-tile kt at cols [kt*C:(kt+1)*C]."""
    r, c = a.shape
    n = r // tile_rows
    return np.ascontiguousarray(
        a.reshape(n, tile_rows, c).transpose(1, 0, 2).reshape(tile_rows, n * c)
    )


def _rope_tables_T(position_ids):
    """cos/sin tables in [d, seq] layout; sin rows 0:64 pre-negated."""
    pos = np.asarray(position_ids).reshape(-1).astype(np.float64)
    inv_freq = 1.0 / (ROPE_THETA ** (np.arange(0, HD, 2, dtype=np.float64) / HD))
    freqs = np.outer(pos, inv_freq)                      # [Q, 64]
    emb = np.concatenate([freqs, freqs], axis=-1)        # [Q, HD]
    cosT = (np.cos(emb).T / W8SCALE).astype(np.float32)  # [128, Q]
    sinT = (np.sin(emb).T / W8SCALE).astype(np.float32)
    sinT[:64, :] = -sinT[:64, :]
    return cosT, sinT


def kernel(hidden_states, attention_mask, position_ids, past_k, past_v,
           Wq, Wk, Wv, Wo):
    global LAST_RESULTS

    hs = np.asarray(hidden_states, np.float32).reshape(Q, HID)
    mask = np.asarray(attention_mask, np.float32).reshape(Q, KV)
    cosT, sinT = _rope_tables_T(position_ids)

    # classify (g, kt) tiles from the additive mask
    keep = mask > -1e8                                   # [Q, KV] True=attend
    kt_lists = []
    boundary = {}
    mtiles = []
    for g in range(N_G):
        lst = []
        for kt in range(N_KT):
            blk = keep[g * GRP:(g + 1) * GRP, kt * 128:(kt + 1) * 128]
            if not blk.any():
                continue
            lst.append(kt)
            if not blk.all():
                boundary[(g, kt)] = len(mtiles)
                mtiles.append(np.ascontiguousarray(blk.T).astype(NPF16))
        kt_lists.append(lst)
    nb = len(mtiles)
    maskb = (np.concatenate(mtiles, axis=1) if nb
             else np.zeros((128, GRP), NPF16))

    scale = 1.0 / math.sqrt(HD)
    hst = _pack_ktiles(np.ascontiguousarray(hs.T)).astype(NPF16)  # [128, 32*1024]

    nc = _build_program(kt_lists, boundary, nb)
    in_maps = []
    for c in range(NCORES):
        qs = slice(c * HPC * HD, (c + 1) * HPC * HD)
        ks = slice(c * HD, (c + 1) * HD)
        # head-major wq: block h = [128, 32*128] = packed Wq.T for head h
        wq_c = np.concatenate(
            [_pack_ktiles(np.ascontiguousarray(
                (Wq[c * HPC * HD + h * HD: c * HPC * HD + (h + 1) * HD, :]
                 * scale).T)).astype(NPF16) for h in range(HPC)],
            axis=1)
        wk_c = _pack_ktiles(np.ascontiguousarray(
            (Wk[ks, :] * W8SCALE).T)).astype(NPF16)                 # [128, 32*128]
        wv_c = _pack_ktiles(
            np.ascontiguousarray(Wv[ks, :].T)).astype(NPF16)
        pkt = np.ascontiguousarray(past_k[0, c].T).astype(NPF16)   # [128, 3072]
        pv = _pack_ktiles(np.ascontiguousarray(past_v[0, c])).astype(NPF16)
        wo_c = _pack_ktiles(
            np.ascontiguousarray(Wo[:, qs].T)).astype(NPF16)       # [128, 4*4096]
        in_maps.append({
            "hst": hst, "wqt": wq_c, "wkt": wk_c, "wvt": wv_c, "pastkt": pkt,
            "pastv": pv, "cost": cosT.astype(NPF16),
            "sint": sinT.astype(NPF16), "maskb": maskb, "wot": wo_c,
        })

    res = run_bass_kernel_spmd(nc, in_maps, list(range(NCORES)))
    LAST_RESULTS = res
    out = np.zeros((Q, HID), np.float32)
    for c in range(NCORES):
        out += res.results[c]["outp"].astype(np.float32)
    return out.reshape(B, Q, HID)


# revision 21
# speedup vs baseline: 1.0024x; 1.0024x over previous
"""Llama GQA attention (B=1, Q=1024, PAST=3072, HID=4096, NH=32, NKV=8, HD=128)
tensor-parallel over heads across 8 NeuronCores.

Per core c: kv head c, query heads 4c..4c+3. Each core computes its partial
o_proj contribution [1024, 4096] (fp16); the host sums the 8 partials.

Per-core design (fp16 datapath, f32 PSUM accumulation):
  - q/k/v projected TRANSPOSED: xT[d, seq] = W-tile.T @ hsT k-tiles, so no
    PE transposes for q/k (v is PE-transposed per 128-tile to [kv, d]).
    RoPE runs on [d, seq] PSUM tiles with 64-partition-shifted DVE ops
    (rotate-half lives on the partition dim); 1/sqrt(HD) is folded into Wq
    host-side so q and k share one rope table pair.
  - projection runs in two waves: k+v k-outer (4 accumulators, paced by
    per-superchunk hs DMA on three engine rings), then q0/q1; q2/q3 matmuls
    are pumped into attention passes 1-2 to fill PE gaps while ACT runs exp.
  - attention in 4 passes of (group g, head-pair): per kv tile kt one
    [128,1024] f32 PSUM scores tile (2 heads sharing the kT LDWEIGHTS), ONE
    exp activation over it, fp16 P. Fully-masked (g,kt) tiles are skipped;
    boundary tiles multiply a 0/1 fp16 mask after exp. Softmax runs without
    max-subtraction (constant shift cancels per row).
  - denominators: DVE fp16 2x-mode accumulation over kt; 1/D via N=1
    matmuls into a seq-on-partitions transpose, one DVE reciprocal
    [128,32], row-ified through identity matmuls and broadcast by a K=1
    matmul (no ACT involvement, so no table-set switches).
  - o_proj: out[seq, hid] tiles; per st the 4 attnT head-slice LDWEIGHTS
    are shared across all 8 hid blocks (8-bank PSUM ring); drains alternate
    DVE/ACT and the last head round pipelines drain+DMA per block.
"""

import math
import numpy as np
import ml_dtypes

import bass_rust
import concourse.bass as bass
import concourse.mybir as mybir
import concourse.tile as tile
from concourse.vector_clock import ScopedClock
from concourse.masks import make_identity
from concourse.bass_utils import run_bass_kernel_spmd

# ---------------------------------------------------------------------------
# Workaround: walrus in this image rejects >1 sem wait on CTRL-class
# instructions (Drain/NoOp). TileContext's tail drain waits on every touched
# logical processor. Split the waits across preceding sync-engine nops.
MAX_WAITS = 1


def _split_waits(nc, inst):
    si = inst.ins.sync_info
    if si is None:
        return
    waits = list(si.on_wait)
    if len(waits) <= MAX_WAITS:
        return
    inst.ins.sync_info = bass_rust.SyncInfo(
        on_wait=waits[:MAX_WAITS], on_update=list(si.on_update)
    )
    rest = waits[MAX_WAITS:]
    while rest:
        extra = nc.sync.nop(nofuse=True)
        extra.ins.sync_info = bass_rust.SyncInfo(on_wait=rest[:MAX_WAITS], on_update=[])
        rest = rest[MAX_WAITS:]


def _drain_and_barrier_split(self, tick_clock, wait_clock):
    nc = self.nc
    carrier = nc.sync.nop(nofuse=True)
    wait_clock.add_sem_waits(carrier.ins, ScopedClock({None: tick_clock.global_clock}))
    _split_waits(nc, carrier)
    nc.sync.drain()
    nc.all_engine_barrier()
    popped = nc._tile_sem_poison_stack.pop()
    assert popped is self._sem_poison
    nc.clear_and_free_semaphores(list(self.sems.allocated().values()))
    nc.all_engine_barrier()


tile.TileContext._drain_and_barrier = _drain_and_barrier_split
# ---------------------------------------------------------------------------

# ---------------------------------------------------------------------------
# General wait-cap legalization: hoist overflow waits onto engine-matched
# NoOps inserted immediately before the offender.
import json as _json

_CTRL_OPS = {"NoOp", "Drain", "EventSemaphore"}
_CAP_CTRL = 1
_CAP_OTHER = 1
_orig_to_json_bytes = bass.Bass.to_json_bytes


def _legalized_to_json_bytes(self, *a, **k):
    raw = _orig_to_json_bytes(self, *a, **k)
    m = _json.loads(raw)
    ctr = [0]
    changed = False
    for fn in m.get("functions", []):
        for blk in fn.get("blocks", []):
            insts = blk.get("instructions", [])
            out = []
            for ins in insts:
                si = ins.get("sync_info")
                if si:
                    waits = si.get("on_wait") or []
                    cap = _CAP_CTRL if ins.get("opcode") in _CTRL_OPS else _CAP_OTHER
                    if len(waits) > cap:
                        changed = True
                        rest = waits[:-cap]
                        si["on_wait"] = waits[-cap:]
                        while rest:
                            ctr[0] += 1
                            out.append({
                                "debug": ins.get("debug", 0),
                                "engine": ins["engine"],
                                "ins": [], "outs": [],
                                "name": f"{ins['name']}_lw{ctr[0]}",
                                "opcode": "NoOp",
                                "sync_info": {"on_wait": rest[:_CAP_CTRL],
                                              "on_update": []},
                            })
                            rest = rest[_CAP_CTRL:]
                out.append(ins)
            blk["instructions"] = out
    if not changed:
        return raw
    return _json.dumps(m).encode()


bass.Bass.to_json_bytes = _legalized_to_json_bytes
# ---------------------------------------------------------------------------


B, Q, PAST, HID = 1, 1024, 3072, 4096
NH, NKV, HD = 32, 8, 128
KV = PAST + Q           # 4096
NCORES = 8
HPC = NH // NCORES      # 4 query heads per core
ROPE_THETA = 10000.0
EXP_SHIFT = -11.0       # constant softmax shift (cancels exactly per row)

F32 = mybir.dt.float32
F16 = mybir.dt.float16
F8 = mybir.dt.float8e4
NPF16 = np.float16
NPF8 = ml_dtypes.float8_e4m3
W8SCALE = 1.0           # fp8 weight rescale (disabled: fp8 failed on HW)

N_KT = KV // 128        # 32 kv tiles
N_HK = HID // 128       # 32 hid k-tiles
GRP = 512
N_G = Q // GRP          # 2 groups
N_PV = PAST // 128      # 24 past-v tiles

LAST_RESULTS = None     # test harness reads exec_time_ns from here


def _build_program(kt_lists, boundary, nb):
    """kt_lists[g] = processed kv tiles for group g (fully-masked skipped);
    boundary[(g, kt)] = slot index into the maskb 0/1 tiles."""
    nc = bass.Bass()
    hst = nc.declare_dram_parameter("hst", [128, N_HK * Q], F16, isOutput=False)
    wqt = nc.declare_dram_parameter("wqt", [128, N_HK * HPC * 128], F16, isOutput=False)
    wkt = nc.declare_dram_parameter("wkt", [128, N_HK * 128], F16, isOutput=False)
    wvt = nc.declare_dram_parameter("wvt", [128, N_HK * 128], F16, isOutput=False)
    pastkt = nc.declare_dram_parameter("pastkt", [128, PAST], F16, isOutput=False)
    pastv = nc.declare_dram_parameter("pastv", [128, PAST], F16, isOutput=False)
    cost = nc.declare_dram_parameter("cost", [128, Q], F16, isOutput=False)
    sint = nc.declare_dram_parameter("sint", [128, Q], F16, isOutput=False)
    maskb = nc.declare_dram_parameter("maskb", [128, max(nb, 1) * GRP], F16,
                                      isOutput=False)
    wot = nc.declare_dram_parameter("wot", [128, HPC * HID], F16, isOutput=False)
    outp = nc.declare_dram_parameter("outp", [Q, HID], F16, isOutput=True)

    with tile.TileContext(nc) as tc:
        with (
            tc.tile_pool(name="const", bufs=1) as cpool,
            tc.tile_pool(name="kvres", bufs=1) as kvpool,
            tc.tile_pool(name="qat", bufs=1) as qat,
            tc.tile_pool(name="tbl", bufs=1) as tbl,
            tc.tile_pool(name="ptp", bufs=3) as ptp,
            tc.tile_pool(name="rt", bufs=1) as rt,
            tc.tile_pool(name="rcp", bufs=2) as rcp,
            tc.tile_pool(name="osb", bufs=4) as osbp,
        ):
            # K^T [128 d, KV]; V packed [128 kv-sub, kt*128 + d]
            kt_sb = kvpool.tile([128, KV], F16)
            v_sb = kvpool.tile([128, KV], F16)
            cos_sb = tbl.tile([128, Q], F16)
            sin_sb = tbl.tile([128, Q], F16)
            mb_sb = tbl.tile([128, max(nb, 1) * GRP], F16)

            qt = [qat.tile([128, Q], F16, tag=f"qt{h}", name=f"qt{h}")
                  for h in range(HPC)]
            atu = [qat.tile([128, Q], F16, tag=f"au{h}", name=f"au{h}")
                   for h in range(HPC)]
            dn = [qat.tile([128, GRP], F16, tag=f"dn{i}", name=f"dn{i}")
                  for i in range(2 * HPC)]

            hsw = tc.alloc_tile_pool(name="hsw", bufs=1)
            hs_sb = hsw.tile([128, N_HK * Q], F16)
            wq_sb = hsw.tile([128, N_HK * HPC * 128], F16)
            wk_sb = hsw.tile([128, N_HK * 128], F16)
            wv_sb = hsw.tile([128, N_HK * 128], F16)
            # DMA order == consumption order, issued round-robin over four
            # engine rings so fixed per-descriptor latencies overlap. wqt is
            # head-major so each q head can start as soon as its block lands.
            rings = [nc.sync, nc.scalar, nc.gpsimd]
            rr = [0]

            def dma(out, in_):
                rings[rr[0] % 3].dma_start(out, in_)
                rr[0] += 1

            SC = 4 * Q
            WH = N_HK * 128          # one head's wq block [128, 4096]
            dma(wk_sb[:], wkt[:])
            dma(hs_sb[:, 0:SC], hst[:, 0:SC])
            dma(wv_sb[:], wvt[:])
            for j in range(1, 4):
                dma(hs_sb[:, j * SC:(j + 1) * SC], hst[:, j * SC:(j + 1) * SC])
            dma(wq_sb[:, 0:WH], wqt[:, 0:WH])
            dma(wq_sb[:, WH:2 * WH], wqt[:, WH:2 * WH])
            for j in range(4, 8):
                dma(hs_sb[:, j * SC:(j + 1) * SC], hst[:, j * SC:(j + 1) * SC])
            dma(wq_sb[:, 2 * WH:3 * WH], wqt[:, 2 * WH:3 * WH])
            dma(wq_sb[:, 3 * WH:4 * WH], wqt[:, 3 * WH:4 * WH])
            dma(cos_sb[:], cost[:])
            dma(sin_sb[:], sint[:])
            dma(kt_sb[:, :PAST], pastkt[:])
            dma(v_sb[:, : N_PV * 128], pastv[:])
            if nb:
                dma(mb_sb[:], maskb[:])

            ident = cpool.tile([128, 128], F16)
            make_identity(nc, ident[:])
            ones_col = cpool.tile([128, 1], F16)
            nc.vector.memset(ones_col[:], 1.0)
            ones_row = cpool.tile([1, 128], F32)
            nc.vector.memset(ones_row[:], 1.0)
            shift_sb = cpool.tile([128, 1], F32)
            nc.vector.memset(shift_sb[:], EXP_SHIFT)
            rcT = cpool.tile([128, 2 * 4 * HPC], F16)   # 1/D, seq on partitions
            # warm the exp table set while DMA streams in
            warm_in = cpool.tile([1, 8], F32)
            warm_out = cpool.tile([1, 8], F32)
            nc.vector.memset(warm_in[:], 0.0)
            nc.scalar.activation(warm_out[:], warm_in[:],
                                 mybir.ActivationFunctionType.Exp)


            def rope_half(dst, ps, g):
                """dst [128 d, 512] f16 <- rope(ps [128 d, 512] f32).

                rows 0:64 of sin_sb are pre-negated:
                  rot[0:64]  = ps[64:128] * sin[0:64]
                  rot[64:128]= ps[0:64]   * sin[64:128]
                """
                c = cos_sb[:, g * GRP:(g + 1) * GRP]
                s = sin_sb[:, g * GRP:(g + 1) * GRP]
                rot = rt.tile([128, GRP], F32, tag="rot", name="rot")
                nc.vector.tensor_mul(rot[0:64, :], ps[64:128, :], s[0:64, :])
                nc.vector.tensor_mul(rot[64:128, :], ps[0:64, :], s[64:128, :])
                cb = rt.tile([128, GRP], F32, tag="cb", name="cb")
                nc.vector.tensor_mul(cb[:], ps[:], c)
                nc.vector.tensor_add(dst, rot[:], cb[:])

            def wk_sl(k):
                return wk_sb[:, k * 128:(k + 1) * 128]

            def wv_sl(k):
                return wv_sb[:, k * 128:(k + 1) * 128]

            def wq_sl(h):
                return lambda k: wq_sb[:, h * N_HK * 128 + k * 128:
                                       h * N_HK * 128 + (k + 1) * 128]

            vts = rt.tile([128, Q], F16, tag="vt", name="vts")

            # ---- wave 1: k & v, k-outer (4 accumulators, DMA-paced) ----
            pj4a = tc.alloc_tile_pool(name="pj4a", bufs=4, space="PSUM")
            tgts = [(wk_sl, 0), (wk_sl, 1), (wv_sl, 0), (wv_sl, 1)]
            p4a = [pj4a.tile([128, GRP], F32, tag="p4a", name=f"p4a_{i}")
                   for i in range(4)]
            for k in range(N_HK):
                for i, (wfn, g) in enumerate(tgts):
                    nc.tensor.matmul(
                        p4a[i][:], wfn(k),
                        hs_sb[:, k * Q + g * GRP: k * Q + (g + 1) * GRP],
                        start=(k == 0), stop=(k == N_HK - 1))
            for g in range(N_G):
                rope_half(kt_sb[:, PAST + g * GRP: PAST + (g + 1) * GRP],
                          p4a[g][:], g)
            for g in range(N_G):
                nc.vector.tensor_copy(vts[:, g * GRP:(g + 1) * GRP],
                                      p4a[2 + g][:])
            pj4a.release()

            # ---- wave 2: q0, q1 sequential halves; ropes trail on DVE ----
            pj4b = tc.alloc_tile_pool(name="pj4b", bufs=4, space="PSUM")
            vtp = tc.alloc_tile_pool(name="vtp", bufs=2, space="PSUM")

            def vt_gen():
                for st in range(Q // 128):
                    tp = vtp.tile([128, 128], F16, tag="vtp", name="tp")
                    nc.tensor.transpose(
                        tp[:], vts[:, st * 128:(st + 1) * 128], ident[:])
                    yield
                    nc.vector.tensor_copy(
                        v_sb[:, (N_PV + st) * 128:(N_PV + st + 1) * 128], tp[:])

            def qk_gen(wslice_fn, dst_fn, pool, ptag):
                for g in range(N_G):
                    ps = pool.tile([128, GRP], F32, tag=ptag, name="qps")
                    for k in range(N_HK):
                        nc.tensor.matmul(
                            ps[:], wslice_fn(k),
                            hs_sb[:, k * Q + g * GRP: k * Q + (g + 1) * GRP],
                            start=(k == 0), stop=(k == N_HK - 1))
                        yield
                    rope_half(dst_fn(g), ps[:], g)

            def q_dst(h):
                return lambda g: qt[h][:, g * GRP:(g + 1) * GRP]

            def pump(gens, n):
                done = 0
                while gens and done < n:
                    try:
                        next(gens[0])
                        done += 1
                    except StopIteration:
                        gens.pop(0)

            w2 = [qk_gen(wq_sl(0), q_dst(0), pj4b, "p4b"), vt_gen(),
                  qk_gen(wq_sl(1), q_dst(1), pj4b, "p4b")]
            pump(w2, 10 ** 9)
            vtp.release()
            pj4b.release()

            # ---- normalization helpers (no ACT engine: 1/D via DVE
            # reciprocal on a seq-on-partitions transpose) ----
            def norm_reduce(g, pool, ttag):
                """rcT[:, g*16 + h*4 + c] = 1 / D(seq c*128+p) for head h."""
                dt = pool.tile([128, GRP], F32, tag=ttag, name="ndt")
                for h in range(HPC):
                    for c in range(4):
                        nc.tensor.matmul(
                            dt[:, h * 4 + c: h * 4 + c + 1],
                            dn[g * HPC + h][:, c * 128:(c + 1) * 128],
                            ones_col[:], start=True, stop=True)
                with nc.allow_low_precision(
                        reason="1/D in fp16: 0.05% rel err, budget is 2e-2"):
                    nc.vector.reciprocal(rcT[:, g * 16:(g + 1) * 16],
                                         dt[:, 0:16])

            def norm_apply_gen(g, pool, ttag):
                for h in range(HPC):
                    t_rc = pool.tile([128, GRP], F32, tag=ttag, name="nrc")
                    for c in range(4):
                        col = g * 16 + h * 4 + c
                        nc.tensor.matmul(
                            t_rc[0:1, c * 128:(c + 1) * 128],
                            rcT[:, col:col + 1], ident[:],
                            start=True, stop=True)
                        yield
                    rc_sb = rcp.tile([1, GRP], F32, tag="rc", name="rcs")
                    nc.vector.tensor_copy(rc_sb[:], t_rc[0:1, :])
                    t_bc = pool.tile([128, GRP], F32, tag=ttag, name="nbc")
                    nc.tensor.matmul(t_bc[:], ones_row[:], rc_sb[:],
                                     start=True, stop=True)
                    yield
                    nc.vector.tensor_mul(
                        atu[h][:, g * GRP:(g + 1) * GRP],
                        atu[h][:, g * GRP:(g + 1) * GRP], t_bc[:])

            # ---- attention passes ----
            pps = tc.alloc_tile_pool(name="pps", bufs=2, space="PSUM")
            scp = tc.alloc_tile_pool(name="scp", bufs=2, space="PSUM")
            att = tc.alloc_tile_pool(name="att", bufs=2, space="PSUM")

            def run_pass(g, ha, hb, bg, per_iter):
                kts = kt_lists[g]
                acc = [att.tile([128, GRP], F32, tag="acc",
                                name=f"acc{g}_{ha}_{jj}") for jj in range(2)]
                for i, kt in enumerate(kts):
                    s_ps = scp.tile([128, 2 * GRP], F32, tag="sc", name="sps")
                    for j, hh in enumerate((ha, hb)):
                        nc.tensor.matmul(
                            s_ps[:, j * GRP:(j + 1) * GRP],
                            kt_sb[:, kt * 128:(kt + 1) * 128],
                            qt[hh][:, g * GRP:(g + 1) * GRP],
                            start=True, stop=True)
                    pump(bg, per_iter)
                    pt = ptp.tile([128, 2 * GRP], F16, tag="pt", name="pt")
                    nc.scalar.activation(
                        pt[:], s_ps[:], mybir.ActivationFunctionType.Exp,
                        bias=shift_sb[:], scale=1.0)
                    sl = boundary.get((g, kt))
                    if sl is not None:
                        for j in range(2):
                            nc.vector.tensor_mul(
                                pt[:, j * GRP:(j + 1) * GRP],
                                pt[:, j * GRP:(j + 1) * GRP],
                                mb_sb[:, sl * GRP:(sl + 1) * GRP])
                    for j, hh in enumerate((ha, hb)):
                        half = pt[:, j * GRP:(j + 1) * GRP]
                        d = dn[g * HPC + hh]
                        if i == 0:
                            nc.vector.tensor_copy(d[:], half)
                        else:
                            nc.vector.tensor_add(d[:], d[:], half)
                        nc.tensor.matmul(
                            acc[j][:], v_sb[:, kt * 128:(kt + 1) * 128],
                            half, start=(i == 0), stop=(i == len(kts) - 1))
                for j, hh in enumerate((ha, hb)):
                    nc.vector.tensor_copy(
                        atu[hh][:, g * GRP:(g + 1) * GRP], acc[j][:])

            pending = [qk_gen(wq_sl(2), q_dst(2), pps, "p"),
                       qk_gen(wq_sl(3), q_dst(3), pps, "p")]
            run_pass(0, 0, 1, pending, 3)
            run_pass(1, 0, 1, pending, 3)
            pump(pending, 10 ** 9)
            # hs/wq/wkv no longer needed; free the space and stream in Wo
            hsw.release()
            wop = tc.alloc_tile_pool(name="wo", bufs=1)
            wo_sb = wop.tile([128, HPC * HID], F16)
            for i in range(8):
                s, e = i * (HPC * HID // 8), (i + 1) * (HPC * HID // 8)
                nc.sync.dma_start(wo_sb[:, s:e], wot[:, s:e])

            run_pass(0, 2, 3, [], 0)
            norm_reduce(0, pps, "p")
            na0 = [norm_apply_gen(0, pps, "p")]
            run_pass(1, 2, 3, na0, 1)
            pump(na0, 10 ** 9)
            att.release()
            scp.release()
            pps.release()

            # ---- tail: o_proj (LDW shared across the 8 n-blocks) ----
            tailp = tc.alloc_tile_pool(name="tailp", bufs=8, space="PSUM")

            def oproj_st(st, bg):
                tiles = [tailp.tile([128, GRP], F32, tag="o", name=f"o{n}")
                         for n in range(8)]
                for h in range(HPC):
                    for n in range(8):
                        nc.tensor.matmul(
                            tiles[n][:], atu[h][:, st * 128:(st + 1) * 128],
                            wo_sb[:, h * HID + n * GRP: h * HID + (n + 1) * GRP],
                            start=(h == 0), stop=(h == HPC - 1))
                        if h == HPC - 1:
                            ob = osbp.tile([128, GRP], F16, tag="ob", name="ob")
                            if (st + n) % 2 == 0:
                                nc.vector.tensor_copy(ob[:], tiles[n][:])
                            else:
                                nc.scalar.activation(
                                    ob[:], tiles[n][:],
                                    mybir.ActivationFunctionType.Copy)
                            nc.sync.dma_start(
                                outp[st * 128:(st + 1) * 128,
                                     n * GRP:(n + 1) * GRP], ob[:])
                    pump(bg, 2)

            oproj_st(0, [])
            norm_reduce(1, tailp, "o")
            na1 = [norm_apply_gen(1, tailp, "o")]
            oproj_st(1, na1)
            oproj_st(2, na1)
            pump(na1, 10 ** 9)
            for st in range(3, 8):
                oproj_st(st, [])
            tailp.release()
            wop.release()
    return nc


def _pack_ktiles(a, tile_rows=128):
    """[R, C] -> [128, (R//128)*C] with k-tile kt at cols [kt*C:(kt+1)*C]."""
    r, c = a.shape
    n = r // tile_rows
    return np.ascontiguousarray(
        a.reshape(n, tile_rows, c).transpose(1, 0, 2).reshape(tile_rows, n * c)
    )


def _rope_tables_T(position_ids):
    """cos/sin tables in [d, seq] layout; sin rows 0:64 pre-negated."""
    pos = np.asarray(position_ids).reshape(-1).astype(np.float64)
    inv_freq = 1.0 / (ROPE_THETA ** (np.arange(0, HD, 2, dtype=np.float64) / HD))
    freqs = np.outer(pos, inv_freq)                      # [Q, 64]
    emb = np.concatenate([freqs, freqs], axis=-1)        # [Q, HD]
    cosT = (np.cos(emb).T / W8SCALE).astype(np.float32)  # [128, Q]
    sinT = (np.sin(emb).T / W8SCALE).astype(np.float32)
    sinT[:64, :] = -sinT[:64, :]
    return cosT, sinT


def kernel(hidden_states, attention_mask, position_ids, past_k, past_v,
           Wq, Wk, Wv, Wo):
    global LAST_RESULTS

    hs = np.asarray(hidden_states, np.float32).reshape(Q, HID)
    mask = np.asarray(attention_mask, np.float32).reshape(Q, KV)
    cosT, sinT = _rope_tables_T(position_ids)

    # classify (g, kt) tiles from the additive mask
    keep = mask > -1e8                                   # [Q, KV] True=attend
    kt_lists = []
    boundary = {}
    mtiles = []
    for g in range(N_G):
        lst = []
        for kt in range(N_KT):
            blk = keep[g * GRP:(g + 1) * GRP, kt * 128:(kt + 1) * 128]
            if not blk.any():
                continue
            lst.append(kt)
            if not blk.all():
                boundary[(g, kt)] = len(mtiles)
                mtiles.append(np.ascontiguousarray(blk.T).astype(NPF16))
        kt_lists.append(lst)
    nb = len(mtiles)
    maskb = (np.concatenate(mtiles, axis=1) if nb
             else np.zeros((128, GRP), NPF16))

    scale = 1.0 / math.sqrt(HD)
    hst = _pack_ktiles(np.ascontiguousarray(hs.T)).astype(NPF16)  # [128, 32*1024]

    nc = _build_program(kt_lists, boundary, nb)
    in_maps = []
    for c in range(NCORES):
        qs = slice(c * HPC * HD, (c + 1) * HPC * HD)
        ks = slice(c * HD, (c + 1) * HD)
        # head-major wq: block h = [128, 32*128] = packed Wq.T for head h
        wq_c = np.concatenate(
            [_pack_ktiles(np.ascontiguousarray(
                (Wq[c * HPC * HD + h * HD: c * HPC * HD + (h + 1) * HD, :]
                 * scale).T)).astype(NPF16) for h in range(HPC)],
            axis=1)
        wk_c = _pack_ktiles(np.ascontiguousarray(
            (Wk[ks, :] * W8SCALE).T)).astype(NPF16)                 # [128, 32*128]
        wv_c = _pack_ktiles(
            np.ascontiguousarray(Wv[ks, :].T)).astype(NPF16)
        pkt = np.ascontiguousarray(past_k[0, c].T).astype(NPF16)   # [128, 3072]
        pv = _pack_ktiles(np.ascontiguousarray(past_v[0, c])).astype(NPF16)
        wo_c = _pack_ktiles(
            np.ascontiguousarray(Wo[:, qs].T)).astype(NPF16)       # [128, 4*4096]
        in_maps.append({
            "hst": hst, "wqt": wq_c, "wkt": wk_c, "wvt": wv_c, "pastkt": pkt,
            "pastv": pv, "cost": cosT.astype(NPF16),
            "sint": sinT.astype(NPF16), "maskb": maskb, "wot": wo_c,
        })

    res = run_bass_kernel_spmd(nc, in_maps, list(range(NCORES)))
    LAST_RESULTS = res
    out = np.zeros((Q, HID), np.float32)
    for c in range(NCORES):
        out += res.results[c]["outp"].astype(np.float32)
    return out.reshape(B, Q, HID)


# revision 24
# speedup vs baseline: 1.0248x; 1.0224x over previous
"""Llama GQA attention (B=1, Q=1024, PAST=3072, HID=4096, NH=32, NKV=8, HD=128)
tensor-parallel over heads across 8 NeuronCores.

Per core c: kv head c, query heads 4c..4c+3. Each core computes its partial
o_proj contribution [1024, 4096] (fp16); the host sums the 8 partials.

Per-core design (fp16 datapath, f32 PSUM accumulation):
  - q/k/v projected TRANSPOSED: xT[d, seq] = W-tile.T @ hsT k-tiles, so no
    PE transposes for q/k (v is PE-transposed per 128-tile to [kv, d]).
    RoPE runs on [d, seq] PSUM tiles with 64-partition-shifted DVE ops
    (rotate-half lives on the partition dim); 1/sqrt(HD) is folded into Wq
    host-side so q and k share one rope table pair.
  - projection runs in two waves: k+v k-outer (4 accumulators, paced by
    per-superchunk hs DMA on three engine rings), then q0/q1; q2/q3 matmuls
    are pumped into attention passes 1-2 to fill PE gaps while ACT runs exp.
  - attention in 4 passes of (group g, head-pair): per kv tile kt one
    [128,1024] f32 PSUM scores tile (2 heads sharing the kT LDWEIGHTS), ONE
    exp activation over it, fp16 P. Fully-masked (g,kt) tiles are skipped;
    boundary tiles multiply a 0/1 fp16 mask after exp. Softmax runs without
    max-subtraction (constant shift cancels per row).
  - denominators: DVE fp16 2x-mode accumulation over kt; 1/D via N=1
    matmuls into a seq-on-partitions transpose, one DVE reciprocal
    [128,32], row-ified through identity matmuls and broadcast by a K=1
    matmul (no ACT involvement, so no table-set switches).
  - o_proj: out[seq, hid] tiles; per st the 4 attnT head-slice LDWEIGHTS
    are shared across all 8 hid blocks (8-bank PSUM ring); drains alternate
    DVE/ACT and the last head round pipelines drain+DMA per block.
"""

import math
import numpy as np
import ml_dtypes

import bass_rust
import concourse.bass as bass
import concourse.mybir as mybir
import concourse.tile as tile
from concourse.vector_clock import ScopedClock
from concourse.masks import make_identity
from concourse.bass_utils import run_bass_kernel_spmd

# ---------------------------------------------------------------------------
# Workaround: walrus in this image rejects >1 sem wait on CTRL-class
# instructions (Drain/NoOp). TileContext's tail drain waits on every touched
# logical processor. Split the waits across preceding sync-engine nops.
MAX_WAITS = 1


def _split_waits(nc, inst):
    si = inst.ins.sync_info
    if si is None:
        return
    waits = list(si.on_wait)
    if len(waits) <= MAX_WAITS:
        return
    inst.ins.sync_info = bass_rust.SyncInfo(
        on_wait=waits[:MAX_WAITS], on_update=list(si.on_update)
    )
    rest = waits[MAX_WAITS:]
    while rest:
        extra = nc.sync.nop(nofuse=True)
        extra.ins.sync_info = bass_rust.SyncInfo(on_wait=rest[:MAX_WAITS], on_update=[])
        rest = rest[MAX_WAITS:]


def _drain_and_barrier_split(self, tick_clock, wait_clock):
    nc = self.nc
    carrier = nc.sync.nop(nofuse=True)
    wait_clock.add_sem_waits(carrier.ins, ScopedClock({None: tick_clock.global_clock}))
    _split_waits(nc, carrier)
    nc.sync.drain()
    nc.all_engine_barrier()
    popped = nc._tile_sem_poison_stack.pop()
    assert popped is self._sem_poison
    nc.clear_and_free_semaphores(list(self.sems.allocated().values()))
    nc.all_engine_barrier()


tile.TileContext._drain_and_barrier = _drain_and_barrier_split
# ---------------------------------------------------------------------------

# ---------------------------------------------------------------------------
# General wait-cap legalization: hoist overflow waits onto engine-matched
# NoOps inserted immediately before the offender.
import json as _json

_CTRL_OPS = {"NoOp", "Drain", "EventSemaphore"}
_CAP_CTRL = 1
_CAP_OTHER = 1
_orig_to_json_bytes = bass.Bass.to_json_bytes


def _legalized_to_json_bytes(self, *a, **k):
    raw = _orig_to_json_bytes(self, *a, **k)
    m = _json.loads(raw)
    ctr = [0]
    changed = False
    for fn in m.get("functions", []):
        for blk in fn.get("blocks", []):
            insts = blk.get("instructions", [])
            out = []
            for ins in insts:
                si = ins.get("sync_info")
                if si:
                    waits = si.get("on_wait") or []
                    cap = _CAP_CTRL if ins.get("opcode") in _CTRL_OPS else _CAP_OTHER
                    if len(waits) > cap:
                        changed = True
                        rest = waits[:-cap]
                        si["on_wait"] = waits[-cap:]
                        while rest:
                            ctr[0] += 1
                            out.append({
                                "debug": ins.get("debug", 0),
                                "engine": ins["engine"],
                                "ins": [], "outs": [],
                                "name": f"{ins['name']}_lw{ctr[0]}",
                                "opcode": "NoOp",
                                "sync_info": {"on_wait": rest[:_CAP_CTRL],
                                              "on_update": []},
                            })
                            rest = rest[_CAP_CTRL:]
                out.append(ins)
            blk["instructions"] = out
    if not changed:
        return raw
    return _json.dumps(m).encode()


bass.Bass.to_json_bytes = _legalized_to_json_bytes
# ---------------------------------------------------------------------------


B, Q, PAST, HID = 1, 1024, 3072, 4096
NH, NKV, HD = 32, 8, 128
KV = PAST + Q           # 4096
NCORES = 8
HPC = NH // NCORES      # 4 query heads per core
ROPE_THETA = 10000.0
EXP_SHIFT = -11.0       # constant softmax shift (cancels exactly per row)

F32 = mybir.dt.float32
F16 = mybir.dt.float16
F8 = mybir.dt.float8e4
NPF16 = np.float16
NPF8 = ml_dtypes.float8_e4m3
W8SCALE = 1.0           # fp8 weight rescale (disabled: fp8 failed on HW)

N_KT = KV // 128        # 32 kv tiles
N_HK = HID // 128       # 32 hid k-tiles
GRP = 512
N_G = Q // GRP          # 2 groups
N_PV = PAST // 128      # 24 past-v tiles

LAST_RESULTS = None     # test harness reads exec_time_ns from here


def _build_program(kt_lists, boundary, nb):
    """kt_lists[g] = processed kv tiles for group g (fully-masked skipped);
    boundary[(g, kt)] = slot index into the maskb 0/1 tiles."""
    nc = bass.Bass()
    hst = nc.declare_dram_parameter("hst", [128, N_HK * Q], F16, isOutput=False)
    wqt = nc.declare_dram_parameter("wqt", [128, N_HK * HPC * 128], F16, isOutput=False)
    wkt = nc.declare_dram_parameter("wkt", [128, N_HK * 128], F16, isOutput=False)
    wvt = nc.declare_dram_parameter("wvt", [128, N_HK * 128], F16, isOutput=False)
    pastkt = nc.declare_dram_parameter("pastkt", [128, PAST], F16, isOutput=False)
    pastv = nc.declare_dram_parameter("pastv", [128, PAST], F16, isOutput=False)
    cost = nc.declare_dram_parameter("cost", [128, Q], F16, isOutput=False)
    sint = nc.declare_dram_parameter("sint", [128, Q], F16, isOutput=False)
    maskb = nc.declare_dram_parameter("maskb", [128, max(nb, 1) * GRP], F16,
                                      isOutput=False)
    wot = nc.declare_dram_parameter("wot", [128, HPC * HID], F16, isOutput=False)
    outp = nc.declare_dram_parameter("outp", [Q, HID], F16, isOutput=True)

    with tile.TileContext(nc) as tc:
        with (
            tc.tile_pool(name="const", bufs=1) as cpool,
            tc.tile_pool(name="kvres", bufs=1) as kvpool,
            tc.tile_pool(name="qat", bufs=1) as qat,
            tc.tile_pool(name="tbl", bufs=1) as tbl,
            tc.tile_pool(name="ptp", bufs=3) as ptp,
            tc.tile_pool(name="rt", bufs=1) as rt,
            tc.tile_pool(name="rcp", bufs=2) as rcp,
            tc.tile_pool(name="osb", bufs=4) as osbp,
        ):
            # K^T [128 d, KV]; V packed [128 kv-sub, kt*128 + d]
            kt_sb = kvpool.tile([128, KV], F16)
            v_sb = kvpool.tile([128, KV], F16)
            cos_sb = tbl.tile([128, Q], F16)
            sin_sb = tbl.tile([128, Q], F16)
            mb_sb = tbl.tile([128, max(nb, 1) * GRP], F16)

            qt = [qat.tile([128, Q], F16, tag=f"qt{h}", name=f"qt{h}")
                  for h in range(HPC)]
            atu = [qat.tile([128, Q], F16, tag=f"au{h}", name=f"au{h}")
                   for h in range(HPC)]
            dn = [qat.tile([128, GRP], F16, tag=f"dn{i}", name=f"dn{i}")
                  for i in range(2 * HPC)]

            hsw = tc.alloc_tile_pool(name="hsw", bufs=1)
            hs_sb = hsw.tile([128, N_HK * Q], F16)
            wq_sb = hsw.tile([128, N_HK * HPC * 128], F16)
            wk_sb = hsw.tile([128, N_HK * 128], F16)
            wv_sb = hsw.tile([128, N_HK * 128], F16)
            # DMA order == consumption order, issued round-robin over four
            # engine rings so fixed per-descriptor latencies overlap. wqt is
            # head-major so each q head can start as soon as its block lands.
            rings = [nc.sync, nc.scalar, nc.gpsimd]
            rr = [0]

            def dma(out, in_):
                rings[rr[0] % 3].dma_start(out, in_)
                rr[0] += 1

            SC = 4 * Q
            WH = N_HK * 128          # one head's wq block [128, 4096]
            WC = WH // 4             # 8-ktile chunk of one head's wq
            dma(wk_sb[:], wkt[:])
            dma(hs_sb[:, 0:SC], hst[:, 0:SC])
            dma(wv_sb[:], wvt[:])
            for c in range(4):       # wq chunks paced against hs superchunks
                for h in (0, 1):
                    dma(wq_sb[:, h * WH + c * WC: h * WH + (c + 1) * WC],
                        wqt[:, h * WH + c * WC: h * WH + (c + 1) * WC])
                for j in (2 * c + 1, 2 * c + 2):
                    if j < 8:
                        dma(hs_sb[:, j * SC:(j + 1) * SC],
                            hst[:, j * SC:(j + 1) * SC])
            dma(hs_sb[:, 7 * SC:8 * SC], hst[:, 7 * SC:8 * SC])
            dma(wq_sb[:, 2 * WH:3 * WH], wqt[:, 2 * WH:3 * WH])
            dma(wq_sb[:, 3 * WH:4 * WH], wqt[:, 3 * WH:4 * WH])
            dma(cos_sb[:], cost[:])
            dma(sin_sb[:], sint[:])
            dma(kt_sb[:, :PAST], pastkt[:])
            dma(v_sb[:, : N_PV * 128], pastv[:])
            if nb:
                dma(mb_sb[:], maskb[:])

            ident = cpool.tile([128, 128], F16)
            make_identity(nc, ident[:])
            ones_col = cpool.tile([128, 1], F16)
            nc.vector.memset(ones_col[:], 1.0)
            ones_row = cpool.tile([1, 128], F32)
            nc.vector.memset(ones_row[:], 1.0)
            shift_sb = cpool.tile([128, 1], F32)
            nc.vector.memset(shift_sb[:], EXP_SHIFT)
            rcT = cpool.tile([128, 2 * 4 * HPC], F16)   # 1/D, seq on partitions
            # warm the exp table set while DMA streams in
            warm_in = cpool.tile([1, 8], F32)
            warm_out = cpool.tile([1, 8], F32)
            nc.vector.memset(warm_in[:], 0.0)
            nc.scalar.activation(warm_out[:], warm_in[:],
                                 mybir.ActivationFunctionType.Exp)


            def rope_half(dst, ps, g):
                """dst [128 d, 512] f16 <- rope(ps [128 d, 512] f32).

                rows 0:64 of sin_sb are pre-negated:
                  rot[0:64]  = ps[64:128] * sin[0:64]
                  rot[64:128]= ps[0:64]   * sin[64:128]
                """
                c = cos_sb[:, g * GRP:(g + 1) * GRP]
                s = sin_sb[:, g * GRP:(g + 1) * GRP]
                rot = rt.tile([128, GRP], F32, tag="rot", name="rot")
                nc.vector.tensor_mul(rot[0:64, :], ps[64:128, :], s[0:64, :])
                nc.vector.tensor_mul(rot[64:128, :], ps[0:64, :], s[64:128, :])
                cb = rt.tile([128, GRP], F32, tag="cb", name="cb")
                nc.vector.tensor_mul(cb[:], ps[:], c)
                nc.vector.tensor_add(dst, rot[:], cb[:])

            def wk_sl(k):
                return wk_sb[:, k * 128:(k + 1) * 128]

            def wv_sl(k):
                return wv_sb[:, k * 128:(k + 1) * 128]

            def wq_sl(h):
                return lambda k: wq_sb[:, h * N_HK * 128 + k * 128:
                                       h * N_HK * 128 + (k + 1) * 128]

            vts = rt.tile([128, Q], F16, tag="vt", name="vts")

            # ---- single k-outer wave: k, v, q0, q1 (8 accumulators,
            # paced by the interleaved hs/wq DMA stream) ----
            pj8 = tc.alloc_tile_pool(name="pj8", bufs=8, space="PSUM")
            tgts = [(wk_sl, 0), (wk_sl, 1), (wv_sl, 0), (wv_sl, 1),
                    (wq_sl(0), 0), (wq_sl(0), 1), (wq_sl(1), 0), (wq_sl(1), 1)]
            p8 = [pj8.tile([128, GRP], F32, tag="p8", name=f"p8_{i}")
                  for i in range(8)]
            for k in range(N_HK):
                for i, (wfn, g) in enumerate(tgts):
                    nc.tensor.matmul(
                        p8[i][:], wfn(k),
                        hs_sb[:, k * Q + g * GRP: k * Q + (g + 1) * GRP],
                        start=(k == 0), stop=(k == N_HK - 1))
            # pass-1-critical ropes straight from PSUM
            rope_half(qt[0][:, 0:GRP], p8[4][:], 0)
            rope_half(qt[1][:, 0:GRP], p8[6][:], 0)
            # stage the remaining accumulators to f16 SBUF so the PSUM pool
            # frees fast; their ropes trail into pass 1 (k is only needed
            # from kv tile 24 onwards, q g1 only from pass 2)
            for g in range(N_G):
                nc.vector.tensor_copy(vts[:, g * GRP:(g + 1) * GRP],
                                      p8[2 + g][:])
            stg = rt.tile([128, 4 * GRP], F16, tag="stg", name="stg")
            nc.vector.tensor_copy(stg[:, 0:GRP], p8[0][:])
            nc.vector.tensor_copy(stg[:, GRP:2 * GRP], p8[1][:])
            nc.vector.tensor_copy(stg[:, 2 * GRP:3 * GRP], p8[5][:])
            nc.vector.tensor_copy(stg[:, 3 * GRP:4 * GRP], p8[7][:])
            pj8.release()

            vtp = tc.alloc_tile_pool(name="vtp", bufs=2, space="PSUM")
            for st in range(Q // 128):
                tp = vtp.tile([128, 128], F16, tag="vtp", name="tp")
                nc.tensor.transpose(
                    tp[:], vts[:, st * 128:(st + 1) * 128], ident[:])
                nc.vector.tensor_copy(
                    v_sb[:, (N_PV + st) * 128:(N_PV + st + 1) * 128], tp[:])
            vtp.release()

            def qk_gen(wslice_fn, dst_fn, pool, ptag):
                for g in range(N_G):
                    ps = pool.tile([128, GRP], F32, tag=ptag, name="qps")
                    for k in range(N_HK):
                        nc.tensor.matmul(
                            ps[:], wslice_fn(k),
                            hs_sb[:, k * Q + g * GRP: k * Q + (g + 1) * GRP],
                            start=(k == 0), stop=(k == N_HK - 1))
                        yield
                    rope_half(dst_fn(g), ps[:], g)

            def q_dst(h):
                return lambda g: qt[h][:, g * GRP:(g + 1) * GRP]

            def pump(gens, n):
                done = 0
                while gens and done < n:
                    try:
                        next(gens[0])
                        done += 1
                    except StopIteration:
                        gens.pop(0)

            # ---- normalization helpers (no ACT engine: 1/D via DVE
            # reciprocal on a seq-on-partitions transpose) ----
            def norm_reduce(g, pool, ttag):
                """rcT[:, g*16 + h*4 + c] = 1 / D(seq c*128+p) for head h."""
                dt = pool.tile([128, GRP], F32, tag=ttag, name="ndt")
                for h in range(HPC):
                    for c in range(4):
                        nc.tensor.matmul(
                            dt[:, h * 4 + c: h * 4 + c + 1],
                            dn[g * HPC + h][:, c * 128:(c + 1) * 128],
                            ones_col[:], start=True, stop=True)
                with nc.allow_low_precision(
                        reason="1/D in fp16: 0.05% rel err, budget is 2e-2"):
                    nc.vector.reciprocal(rcT[:, g * 16:(g + 1) * 16],
                                         dt[:, 0:16])

            def norm_apply_gen(g, pool, ttag):
                for h in range(HPC):
                    t_rc = pool.tile([128, GRP], F32, tag=ttag, name="nrc")
                    for c in range(4):
                        col = g * 16 + h * 4 + c
                        nc.tensor.matmul(
                            t_rc[0:1, c * 128:(c + 1) * 128],
                            rcT[:, col:col + 1], ident[:],
                            start=True, stop=True)
                        yield
                    rc_sb = rcp.tile([1, GRP], F32, tag="rc", name="rcs")
                    nc.vector.tensor_copy(rc_sb[:], t_rc[0:1, :])
                    t_bc = pool.tile([128, GRP], F32, tag=ttag, name="nbc")
                    nc.tensor.matmul(t_bc[:], ones_row[:], rc_sb[:],
                                     start=True, stop=True)
                    yield
                    nc.vector.tensor_mul(
                        atu[h][:, g * GRP:(g + 1) * GRP],
                        atu[h][:, g * GRP:(g + 1) * GRP], t_bc[:])

            # ---- attention passes ----
            pps = tc.alloc_tile_pool(name="pps", bufs=2, space="PSUM")
            scp = tc.alloc_tile_pool(name="scp", bufs=2, space="PSUM")
            att = tc.alloc_tile_pool(name="att", bufs=2, space="PSUM")

            def run_pass(g, ha, hb, bg, per_iter):
                kts = kt_lists[g]
                acc = [att.tile([128, GRP], F32, tag="acc",
                                name=f"acc{g}_{ha}_{jj}") for jj in range(2)]
                for i, kt in enumerate(kts):
                    s_ps = scp.tile([128, 2 * GRP], F32, tag="sc", name="sps")
                    for j, hh in enumerate((ha, hb)):
                        nc.tensor.matmul(
                            s_ps[:, j * GRP:(j + 1) * GRP],
                            kt_sb[:, kt * 128:(kt + 1) * 128],
                            qt[hh][:, g * GRP:(g + 1) * GRP],
                            start=True, stop=True)
                    pump(bg, per_iter)
                    pt = ptp.tile([128, 2 * GRP], F16, tag="pt", name="pt")
                    nc.scalar.activation(
                        pt[:], s_ps[:], mybir.ActivationFunctionType.Exp,
                        bias=shift_sb[:], scale=1.0)
                    sl = boundary.get((g, kt))
                    if sl is not None:
                        for j in range(2):
                            nc.vector.tensor_mul(
                                pt[:, j * GRP:(j + 1) * GRP],
                                pt[:, j * GRP:(j + 1) * GRP],
                                mb_sb[:, sl * GRP:(sl + 1) * GRP])
                    for j, hh in enumerate((ha, hb)):
                        half = pt[:, j * GRP:(j + 1) * GRP]
                        d = dn[g * HPC + hh]
                        if i == 0:
                            nc.vector.tensor_copy(d[:], half)
                        else:
                            nc.vector.tensor_add(d[:], d[:], half)
                        nc.tensor.matmul(
                            acc[j][:], v_sb[:, kt * 128:(kt + 1) * 128],
                            half, start=(i == 0), stop=(i == len(kts) - 1))
                for j, hh in enumerate((ha, hb)):
                    nc.vector.tensor_copy(
                        atu[hh][:, g * GRP:(g + 1) * GRP], acc[j][:])

            def rope_sbuf(dst, src, g):
                """Trailing rope on an f16 SBUF staging tile. The verifier
                bans partition-shifted 2-input ops with both inputs in SBUF,
                so swap halves first with 1-input copies, then stay aligned."""
                c = cos_sb[:, g * GRP:(g + 1) * GRP]
                s = sin_sb[:, g * GRP:(g + 1) * GRP]
                sw = rt.tile([128, GRP], F16, tag="sw", name="sw")
                nc.vector.tensor_copy(sw[0:64, :], src[64:128, :])
                nc.vector.tensor_copy(sw[64:128, :], src[0:64, :])
                rot = rt.tile([128, GRP], F32, tag="rot", name="rot")
                nc.vector.tensor_mul(rot[:], sw[:], s)
                cb = rt.tile([128, GRP], F32, tag="cb", name="cb")
                nc.vector.tensor_mul(cb[:], src, c)
                nc.vector.tensor_add(dst, rot[:], cb[:])

            rope_sbuf(kt_sb[:, PAST:PAST + GRP], stg[:, 0:GRP], 0)
            rope_sbuf(kt_sb[:, PAST + GRP:PAST + 2 * GRP], stg[:, GRP:2 * GRP],
                      1)
            rope_sbuf(qt[0][:, GRP:Q], stg[:, 2 * GRP:3 * GRP], 1)
            rope_sbuf(qt[1][:, GRP:Q], stg[:, 3 * GRP:4 * GRP], 1)
            pending = [qk_gen(wq_sl(2), q_dst(2), pps, "p"),
                       qk_gen(wq_sl(3), q_dst(3), pps, "p")]
            run_pass(0, 0, 1, pending, 3)
            run_pass(1, 0, 1, pending, 3)
            pump(pending, 10 ** 9)
            # hs/wq/wkv no longer needed; free the space and stream in Wo
            hsw.release()
            wop = tc.alloc_tile_pool(name="wo", bufs=1)
            wo_sb = wop.tile([128, HPC * HID], F16)
            for i in range(8):
                s, e = i * (HPC * HID // 8), (i + 1) * (HPC * HID // 8)
                nc.sync.dma_start(wo_sb[:, s:e], wot[:, s:e])

            run_pass(0, 2, 3, [], 0)
            norm_reduce(0, pps, "p")
            na0 = [norm_apply_gen(0, pps, "p")]
            run_pass(1, 2, 3, na0, 1)
            pump(na0, 10 ** 9)
            att.release()
            scp.release()
            pps.release()

            # ---- tail: o_proj (LDW shared across the 8 n-blocks) ----
            tailp = tc.alloc_tile_pool(name="tailp", bufs=8, space="PSUM")

            def oproj_st(st, bg):
                tiles = [tailp.tile([128, GRP], F32, tag="o", name=f"o{n}")
                         for n in range(8)]
                for h in range(HPC):
                    for n in range(8):
                        nc.tensor.matmul(
                            tiles[n][:], atu[h][:, st * 128:(st + 1) * 128],
                            wo_sb[:, h * HID + n * GRP: h * HID + (n + 1) * GRP],
                            start=(h == 0), stop=(h == HPC - 1))
                        if h == HPC - 1:
                            ob = osbp.tile([128, GRP], F16, tag="ob", name="ob")
                            if (st + n) % 2 == 0:
                                nc.vector.tensor_copy(ob[:], tiles[n][:])
                            else:
                                nc.scalar.activation(
                                    ob[:], tiles[n][:],
                                    mybir.ActivationFunctionType.Copy)
                            nc.sync.dma_start(
                                outp[st * 128:(st + 1) * 128,
                                     n * GRP:(n + 1) * GRP], ob[:])
                    pump(bg, 2)

            oproj_st(0, [])
            norm_reduce(1, tailp, "o")
            na1 = [norm_apply_gen(1, tailp, "o")]
            oproj_st(1, na1)
            oproj_st(2, na1)
            pump(na1, 10 ** 9)
            for st in range(3, 8):
                oproj_st(st, [])
            tailp.release()
            wop.release()
    return nc


def _pack_ktiles(a, tile_rows=128):
    """[R, C] -> [128, (R//128)*C] with k-tile kt at cols [kt*C:(kt+1)*C]."""
    r, c = a.shape
    n = r // tile_rows
    return np.ascontiguousarray(
        a.reshape(n, tile_rows, c).transpose(1, 0, 2).reshape(tile_rows, n * c)
    )


def _rope_tables_T(position_ids):
    """cos/sin tables in [d, seq] layout; sin rows 0:64 pre-negated."""
    pos = np.asarray(position_ids).reshape(-1).astype(np.float64)
    inv_freq = 1.0 / (ROPE_THETA ** (np.arange(0, HD, 2, dtype=np.float64) / HD))
    freqs = np.outer(pos, inv_freq)                      # [Q, 64]
    emb = np.concatenate([freqs, freqs], axis=-1)        # [Q, HD]
    cosT = (np.cos(emb).T / W8SCALE).astype(np.float32)  # [128, Q]
    sinT = (np.sin(emb).T / W8SCALE).astype(np.float32)
    sinT[:64, :] = -sinT[:64, :]
    return cosT, sinT


def kernel(hidden_states, attention_mask, position_ids, past_k, past_v,
           Wq, Wk, Wv, Wo):
    global LAST_RESULTS

    hs = np.asarray(hidden_states, np.float32).reshape(Q, HID)
    mask = np.asarray(attention_mask, np.float32).reshape(Q, KV)
    cosT, sinT = _rope_tables_T(position_ids)

    # classify (g, kt) tiles from the additive mask
    keep = mask > -1e8                                   # [Q, KV] True=attend
    kt_lists = []
    boundary = {}
    mtiles = []
    for g in range(N_G):
        lst = []
        for kt in range(N_KT):
            blk = keep[g * GRP:(g + 1) * GRP, kt * 128:(kt + 1) * 128]
            if not blk.any():
                continue
            lst.append(kt)
            if not blk.all():
                boundary[(g, kt)] = len(mtiles)
                mtiles.append(np.ascontiguousarray(blk.T).astype(NPF16))
        kt_lists.append(lst)
    nb = len(mtiles)
    maskb = (np.concatenate(mtiles, axis=1) if nb
             else np.zeros((128, GRP), NPF16))

    scale = 1.0 / math.sqrt(HD)
    hst = _pack_ktiles(np.ascontiguousarray(hs.T)).astype(NPF16)  # [128, 32*1024]

    nc = _build_program(kt_lists, boundary, nb)
    in_maps = []
    for c in range(NCORES):
        qs = slice(c * HPC * HD, (c + 1) * HPC * HD)
        ks = slice(c * HD, (c + 1) * HD)
        # head-major wq: block h = [128, 32*128] = packed Wq.T for head h
        wq_c = np.concatenate(
            [_pack_ktiles(np.ascontiguousarray(
                (Wq[c * HPC * HD + h * HD: c * HPC * HD + (h + 1) * HD, :]
                 * scale).T)).astype(NPF16) for h in range(HPC)],
            axis=1)
        wk_c = _pack_ktiles(np.ascontiguousarray(
            (Wk[ks, :] * W8SCALE).T)).astype(NPF16)                 # [128, 32*128]
        wv_c = _pack_ktiles(
            np.ascontiguousarray(Wv[ks, :].T)).astype(NPF16)
        pkt = np.ascontiguousarray(past_k[0, c].T).astype(NPF16)   # [128, 3072]
        pv = _pack_ktiles(np.ascontiguousarray(past_v[0, c])).astype(NPF16)
        wo_c = _pack_ktiles(
            np.ascontiguousarray(Wo[:, qs].T)).astype(NPF16)       # [128, 4*4096]
        in_maps.append({
            "hst": hst, "wqt": wq_c, "wkt": wk_c, "wvt": wv_c, "pastkt": pkt,
            "pastv": pv, "cost": cosT.astype(NPF16),
            "sint": sinT.astype(NPF16), "maskb": maskb, "wot": wo_c,
        })

    res = run_bass_kernel_spmd(nc, in_maps, list(range(NCORES)))
    LAST_RESULTS = res
    out = np.zeros((Q, HID), np.float32)
    for c in range(NCORES):
        out += res.results[c]["outp"].astype(np.float32)
    return out.reshape(B, Q, HID)
